# revision 17
# baseline (speedup 1.0000x reference)
"""LMS adaptive filter (BaseFilter) on 8 TRN2 NeuronCores.

Algorithm per (batch b, frame f): 64-tap LMS over 416 sequential steps.
  e_t   = d[b, 256f + 32 + t] - sum_k w[k] * x[256f + t + k]
  w     = clip(w + MU * e_t * x[256f + t : +64], +-65535)
The clip is essential: mu*|x_win|^2 ~ 3.2 > 2 makes the recursion
unstable, so w rides the clip rails and the rails keep all float
implementations shadowing each other.
Outputs (overlap-add): d_est = d - e and e, assembled per reference.

Sharding: 4096 frames split 512/core (both batches on every core) ->
1024 independent sequences/core = 8 groups x 128 partitions.

Inner loop = 2 custom fused DVE instructions per step per group:
  LMS_SCAN_DOT : out = scan(add, -(w*x), init=d_t) streamed BACKWARD
                 over the taps, so the final scan element (the complete
                 e_t) lands at EB[:, t]; the 63 partial prefixes spill
                 into EB[t+1 : t+64] and are overwritten by later steps
                 (EB has a 63-column junk tail for the last steps).
  LMS_UPD_CLIP : w = clip((mu*x)*e_t + w, +-65535) in one pass.
"""

import numpy as np

HOP = 256
FRAMELEN = 512
K = 64
WD = 32
MU = 0.05
WMIN, WMAX = -65535.0, 65535.0
B = 2
F = 4096
NC = 8
F_LOC = F // NC              # 512 frames per core
S = (FRAMELEN - K) - WD      # 416 sequential steps
TSTART = (FRAMELEN - HOP) - WD  # 224: first step kept for frames >= 1
TAIL = S - TSTART            # 192 output elements per frame >= 1
SPAN = HOP * (F_LOC - 1) + FRAMELEN  # 131328: x/d elements per core shard
CORE_STRIDE = HOP * F_LOC    # 131072
OUT_LEN = (FRAMELEN - K) + (F - 1) * TAIL  # 786688

# number of groups whose update+clip runs on GpSimd instead of DVE
GP_GROUPS = 2

_CACHE = {}


def _custom_ops():
    """Register the two fused DVE ops (runtime registration; shas computed
    live so the pinned-sha check in dve_table_for_ops passes)."""
    import concourse.dve_ops as dve_ops
    from concourse.dve_ops import DveOp
    from concourse.dve_spec import (
        Spec, Src0, Src1, C0, C1, C2, Zero, scan, AluOp as DveAluOp,
        minn, maxx, lower, _has_src1,
    )
    from concourse.dve_uop import DveOpSpec

    def _ref_scan_dot(in0, in1, s0, s1, imm2):
        c = np.cumsum(-(in0.astype(np.float32) * in1.astype(np.float32)),
                      axis=-1)
        return (np.asarray(s0).reshape(-1, 1) + c).astype(np.float32)

    def _ref_updclip(in0, in1, s0, s1, imm2):
        v = (in0.astype(np.float32) * np.asarray(s0).reshape(-1, 1)
             + in1.astype(np.float32))
        return np.clip(v, imm2, s1).astype(np.float32)

    def _register(name, spec, subdim=False):
        for op in dve_ops.OPS:
            if op.name == name:
                return op
        shas = {}
        for ver in ("v3", "v4"):
            tmp = DveOpSpec(name=name, opcode=1, uops=lower(spec, ver=ver),
                            rd1_en=_has_src1(spec))
            shas[ver] = tmp.sha(ver)
        op = DveOp(name, spec, subdim=subdim, uops_sha=shas)
        dve_ops.OPS.append(op)
        dve_ops.CUSTOM_DVE_SPECS[name] = spec
        dve_ops._SUB_OPCODE_FOR_NAME[name] = (
            dve_ops._CUSTOM_DVE_ROW_BASE + len(dve_ops.OPS) - 1)
        return op

    scan_dot = _register(
        "LMS_SCAN_DOT",
        Spec(body=scan(DveAluOp.ADD, Zero - Src0 * Src1, init=C0),
             reference=_ref_scan_dot),
    )
    updclip = _register(
        "LMS_UPD_CLIP",
        Spec(body=maxx(minn(Src0 * C0 + Src1, C1), C2),
             reference=_ref_updclip),
    )
    return scan_dot, updclip


def _build():
    import concourse.bacc as bacc
    import concourse.tile as tile
    from concourse import mybir
    import concourse.bass as bass

    f32 = mybir.dt.float32
    AluOp = mybir.AluOpType
    SCAN_DOT, UPD_CLIP = _custom_ops()

    nc = bacc.Bacc("TRN2", target_bir_lowering=False)
    x_in = nc.dram_tensor("x", [SPAN], f32, kind="ExternalInput")
    d_in = nc.dram_tensor("d", [B, SPAN], f32, kind="ExternalInput")
    # [kind(0=d_est,1=e)][b][f_local][j] , j <-> step t = TSTART + j
    out_main = nc.dram_tensor("out_main", [2, B, F_LOC, TAIL], f32,
                              kind="ExternalOutput")
    # frame 0 of this core: steps t < TSTART   [kind][b][t]
    out_head = nc.dram_tensor("out_head", [2, B, TSTART], f32,
                              kind="ExternalOutput")

    def rev(ap, offset_add, n):
        # reversed innermost free dim: n elements ending at the AP's origin
        return bass.AP(tensor=ap.tensor, offset=ap.offset + offset_add,
                       ap=[ap.ap[0], [-1, n]])

    with tile.TileContext(nc) as tc:
        with tc.tile_pool(name="p", bufs=1) as pool:
            XF = pool.tile([128, 4, FRAMELEN], f32)    # x frames (slab fg)
            XFMU = pool.tile([128, 4, FRAMELEN], f32)  # MU * x frames
            DB = pool.tile([128, B, 4, S], f32)        # d at step offsets
            W8 = pool.tile([128, B, 4, K], f32)    # weights, all groups
            EB8 = pool.tile([128, B, 4, S], f32)   # e values, all groups
            NS8 = pool.tile([128, B, 4], f32)      # per-step negated dots
            DEST = [pool.tile([128, S], f32, name=f"DEST{g}", tag=f"de{g}")
                    for g in range(8)]

            # partition p, slab fg  ->  frame f_local = fg*128 + p
            nc.sync.dma_start(
                XF[:],
                bass.AP(tensor=x_in, offset=0,
                        ap=[[HOP, 128], [HOP * 128, 4], [1, FRAMELEN]]),
            )
            for b in range(B):
                nc.sync.dma_start(
                    DB[:, b, :, :],
                    bass.AP(tensor=d_in, offset=b * SPAN + WD,
                            ap=[[HOP, 128], [HOP * 128, 4], [1, S]]),
                )
            nc.vector.tensor_scalar_mul(XFMU[:], XF[:], MU)
            nc.vector.memset(W8[:], 0.0)

            PROD = [pool.tile([128, K], f32, name=f"PROD{g}", tag=f"pr{g}")
                    for g in range(8)]
            for t in range(S):
                # two waves (b=0 groups, then b=1 groups): e for a wave is
                # one DVE op; clips of wave 0 overlap wave 1's dots on GpSimd
                for b in range(B):
                    for fg in range(4):
                        g = b * 4 + fg
                        nc.vector.scalar_tensor_tensor(
                            out=PROD[g][:], in0=W8[:, b, fg, :], scalar=-1.0,
                            in1=XF[:, fg, t:t + K],
                            op0=AluOp.mult, op1=AluOp.mult,
                            accum_out=NS8[:, b, fg:fg + 1],
                        )
                    nc.vector.tensor_tensor(
                        out=EB8[:, b, :, t], in0=NS8[:, b, :],
                        in1=DB[:, b, :, t], op=AluOp.add,
                    )
                    for fg in range(4):
                        nc.vector.scalar_tensor_tensor(
                            out=W8[:, b, fg, :], in0=XFMU[:, fg, t:t + K],
                            scalar=EB8[:, b, fg, t:t + 1],
                            in1=W8[:, b, fg, :],
                            op0=AluOp.mult, op1=AluOp.add,
                        )
                    # clip the whole wave in one GpSimd op
                    nc.gpsimd.tensor_scalar(
                        out=W8[:, b, :, :], in0=W8[:, b, :, :],
                        scalar1=WMAX, scalar2=WMIN,
                        op0=AluOp.min, op1=AluOp.max,
                    )

            # d_est = d - e
            for g in range(8):
                b, fg = divmod(g, 4)
                nc.vector.tensor_sub(DEST[g][:], DB[:, b, fg, :],
                                     EB8[:, b, fg, :])

            # outputs: frames >= 1 use steps [TSTART, S); f_local = fg*128+p
            for g in range(8):
                b, fg = divmod(g, 4)
                for kind, src in ((0, DEST[g][:, TSTART:S]),
                                  (1, EB8[:, b, fg, TSTART:S])):
                    nc.sync.dma_start(
                        bass.AP(tensor=out_main,
                                offset=(kind * B + b) * F_LOC * TAIL
                                + fg * 128 * TAIL,
                                ap=[[TAIL, 128], [1, TAIL]]),
                        src,
                    )
            # head: local frame 0 = (fg=0, p=0) -> groups 0 (b=0) and 4 (b=1)
            for b in range(B):
                g = b * 4
                for kind, src in ((0, DEST[g][0:1, 0:TSTART]),
                                  (1, EB8[0:1, b, 0, 0:TSTART])):
                    nc.sync.dma_start(
                        bass.AP(tensor=out_head,
                                offset=(kind * B + b) * TSTART,
                                ap=[[TSTART, 1], [1, TSTART]]),
                        src,
                    )
    nc.finalize()
    return nc


def _get_nc():
    if "nc" not in _CACHE:
        _CACHE["nc"] = _build()
    return _CACHE["nc"]


def run_shards(d, x, trace=False, **kw):
    from concourse.bass_utils import run_bass_kernel_spmd

    nc = _get_nc()
    in_maps = []
    for c in range(NC):
        lo = c * CORE_STRIDE
        in_maps.append({
            "x": np.ascontiguousarray(x[lo:lo + SPAN], dtype=np.float32),
            "d": np.ascontiguousarray(d[:, lo:lo + SPAN], dtype=np.float32),
        })
    return run_bass_kernel_spmd(nc, in_maps, core_ids=list(range(NC)),
                                trace=trace, **kw)


def assemble(results):
    mains = np.stack([r["out_main"] for r in results])  # (8, 2, B, 512, 192)
    head = results[0]["out_head"]                       # (2, B, 224)
    outs = []
    for kind in range(2):
        m = mains[:, kind].transpose(1, 0, 2, 3).reshape(B, F, TAIL)
        o = np.zeros((B, OUT_LEN), np.float32)
        o[:, WD:WD + TSTART] = head[kind]
        o[:, WD + TSTART:FRAMELEN - K] = m[:, 0]
        o[:, FRAMELEN - K:] = m[:, 1:].reshape(B, -1)
        outs.append(o)
    return outs[0], outs[1]


def kernel(d, x):
    res = run_shards(d, x)
    return assemble(res.results)


# revision 18
# speedup vs baseline: 1.4403x; 1.4403x over previous
"""LMS adaptive filter (BaseFilter) on 8 TRN2 NeuronCores.

Algorithm per (batch b, frame f): 64-tap LMS over 416 sequential steps.
  e_t   = d[b, 256f + 32 + t] - sum_k w[k] * x[256f + t + k]
  w     = clip(w + MU * e_t * x[256f + t : +64], +-65535)
The clip is essential: mu*|x_win|^2 ~ 3.2 > 2 makes the recursion
unstable, so w rides the clip rails and the rails keep all float
implementations shadowing each other.
Outputs (overlap-add): d_est = d - e and e, assembled per reference.

Sharding: 4096 frames split 512/core (both batches on every core) ->
1024 independent sequences/core = 8 groups x 128 partitions.

Inner loop = 2 custom fused DVE instructions per step per group:
  LMS_SCAN_DOT : out = scan(add, -(w*x), init=d_t) streamed BACKWARD
                 over the taps, so the final scan element (the complete
                 e_t) lands at EB[:, t]; the 63 partial prefixes spill
                 into EB[t+1 : t+64] and are overwritten by later steps
                 (EB has a 63-column junk tail for the last steps).
  LMS_UPD_CLIP : w = clip((mu*x)*e_t + w, +-65535) in one pass.
"""

import numpy as np

HOP = 256
FRAMELEN = 512
K = 64
WD = 32
MU = 0.05
WMIN, WMAX = -65535.0, 65535.0
B = 2
F = 4096
NC = 8
F_LOC = F // NC              # 512 frames per core
S = (FRAMELEN - K) - WD      # 416 sequential steps
TSTART = (FRAMELEN - HOP) - WD  # 224: first step kept for frames >= 1
TAIL = S - TSTART            # 192 output elements per frame >= 1
SPAN = HOP * (F_LOC - 1) + FRAMELEN  # 131328: x/d elements per core shard
CORE_STRIDE = HOP * F_LOC    # 131072
OUT_LEN = (FRAMELEN - K) + (F - 1) * TAIL  # 786688

# number of groups whose update+clip runs on GpSimd instead of DVE
GP_GROUPS = 2

_CACHE = {}


def _custom_ops():
    """Register the two fused DVE ops (runtime registration; shas computed
    live so the pinned-sha check in dve_table_for_ops passes)."""
    import concourse.dve_ops as dve_ops
    from concourse.dve_ops import DveOp
    from concourse.dve_spec import (
        Spec, Src0, Src1, C0, C1, C2, Zero, scan, AluOp as DveAluOp,
        minn, maxx, lower, _has_src1,
    )
    from concourse.dve_uop import DveOpSpec

    def _ref_scan_dot(in0, in1, s0, s1, imm2):
        c = np.cumsum(-(in0.astype(np.float32) * in1.astype(np.float32)),
                      axis=-1)
        return (np.asarray(s0).reshape(-1, 1) + c).astype(np.float32)

    def _ref_updclip(in0, in1, s0, s1, imm2):
        v = (in0.astype(np.float32) * np.asarray(s0).reshape(-1, 1)
             + in1.astype(np.float32))
        return np.clip(v, imm2, s1).astype(np.float32)

    def _register(name, spec, subdim=False):
        for op in dve_ops.OPS:
            if op.name == name:
                return op
        shas = {}
        for ver in ("v3", "v4"):
            tmp = DveOpSpec(name=name, opcode=1, uops=lower(spec, ver=ver),
                            rd1_en=_has_src1(spec))
            shas[ver] = tmp.sha(ver)
        op = DveOp(name, spec, subdim=subdim, uops_sha=shas)
        dve_ops.OPS.append(op)
        dve_ops.CUSTOM_DVE_SPECS[name] = spec
        dve_ops._SUB_OPCODE_FOR_NAME[name] = (
            dve_ops._CUSTOM_DVE_ROW_BASE + len(dve_ops.OPS) - 1)
        return op

    scan_dot = _register(
        "LMS_SCAN_DOT",
        Spec(body=scan(DveAluOp.ADD, Zero - Src0 * Src1, init=C0),
             reference=_ref_scan_dot),
    )
    updclip = _register(
        "LMS_UPD_CLIP",
        Spec(body=maxx(minn(Src0 * C0 + Src1, C1), C2),
             reference=_ref_updclip),
    )
    return scan_dot, updclip


def _build():
    import concourse.bacc as bacc
    import concourse.tile as tile
    from concourse import mybir
    import concourse.bass as bass

    f32 = mybir.dt.float32
    AluOp = mybir.AluOpType
    SCAN_DOT, UPD_CLIP = _custom_ops()

    nc = bacc.Bacc("TRN2", target_bir_lowering=False)
    x_in = nc.dram_tensor("x", [SPAN], f32, kind="ExternalInput")
    d_in = nc.dram_tensor("d", [B, SPAN], f32, kind="ExternalInput")
    # [kind(0=d_est,1=e)][b][f_local][j] , j <-> step t = TSTART + j
    out_main = nc.dram_tensor("out_main", [2, B, F_LOC, TAIL], f32,
                              kind="ExternalOutput")
    # frame 0 of this core: steps t < TSTART   [kind][b][t]
    out_head = nc.dram_tensor("out_head", [2, B, TSTART], f32,
                              kind="ExternalOutput")

    def rev(ap, offset_add, n):
        # reversed innermost free dim: n elements ending at the AP's origin
        return bass.AP(tensor=ap.tensor, offset=ap.offset + offset_add,
                       ap=[ap.ap[0], [-1, n]])

    with tile.TileContext(nc) as tc:
        with tc.tile_pool(name="p", bufs=1) as pool:
            XF = pool.tile([128, 4, FRAMELEN], f32)    # x frames (slab fg)
            XFMU = pool.tile([128, 4, FRAMELEN], f32)  # MU * x frames
            DB = pool.tile([128, B, 4, S], f32)        # d at step offsets
            W = [pool.tile([128, K], f32, name=f"W{g}", tag=f"w{g}")
                 for g in range(8)]
            EB8 = pool.tile([128, B, 4, S], f32)   # e values, all groups
            NS8 = pool.tile([128, B, 4], f32)      # per-step negated dots
            DEST = [pool.tile([128, S], f32, name=f"DEST{g}", tag=f"de{g}")
                    for g in range(8)]

            # partition p, slab fg  ->  frame f_local = fg*128 + p
            nc.sync.dma_start(
                XF[:],
                bass.AP(tensor=x_in, offset=0,
                        ap=[[HOP, 128], [HOP * 128, 4], [1, FRAMELEN]]),
            )
            for b in range(B):
                nc.sync.dma_start(
                    DB[:, b, :, :],
                    bass.AP(tensor=d_in, offset=b * SPAN + WD,
                            ap=[[HOP, 128], [HOP * 128, 4], [1, S]]),
                )
            nc.vector.tensor_scalar_mul(XFMU[:], XF[:], MU)
            for g in range(8):
                nc.vector.memset(W[g][:], 0.0)

            PROD = [pool.tile([128, K], f32, name=f"PROD{g}", tag=f"pr{g}")
                    for g in range(8)]
            for t in range(S):
                # two waves (b=0 groups, then b=1 groups): e for a wave is
                # one DVE op; clips of wave 0 overlap wave 1's dots on GpSimd
                for g in range(8):
                    b, fg = divmod(g, 4)
                    nc.vector.scalar_tensor_tensor(
                        out=PROD[g][:], in0=W[g][:], scalar=-1.0,
                        in1=XF[:, fg, t:t + K],
                        op0=AluOp.mult, op1=AluOp.mult,
                        accum_out=NS8[:, b, fg:fg + 1],
                    )
                for g in range(8):
                    b, fg = divmod(g, 4)
                    nc.scalar.activation(
                        out=EB8[:, b, fg, t:t + 1], in_=NS8[:, b, fg:fg + 1],
                        func=mybir.ActivationFunctionType.Identity,
                        bias=DB[:, b, fg, t:t + 1], scale=1.0,
                    )
                for g in range(8):
                    b, fg = divmod(g, 4)
                    nc.vector.scalar_tensor_tensor(
                        out=W[g][:], in0=XFMU[:, fg, t:t + K],
                        scalar=EB8[:, b, fg, t:t + 1], in1=W[g][:],
                        op0=AluOp.mult, op1=AluOp.add,
                    )
                for g in range(8):
                    nc.gpsimd.tensor_scalar(
                        out=W[g][:], in0=W[g][:],
                        scalar1=WMAX, scalar2=WMIN,
                        op0=AluOp.min, op1=AluOp.max,
                    )

            # d_est = d - e
            for g in range(8):
                b, fg = divmod(g, 4)
                nc.vector.tensor_sub(DEST[g][:], DB[:, b, fg, :],
                                     EB8[:, b, fg, :])

            # outputs: frames >= 1 use steps [TSTART, S); f_local = fg*128+p
            for g in range(8):
                b, fg = divmod(g, 4)
                for kind, src in ((0, DEST[g][:, TSTART:S]),
                                  (1, EB8[:, b, fg, TSTART:S])):
                    nc.sync.dma_start(
                        bass.AP(tensor=out_main,
                                offset=(kind * B + b) * F_LOC * TAIL
                                + fg * 128 * TAIL,
                                ap=[[TAIL, 128], [1, TAIL]]),
                        src,
                    )
            # head: local frame 0 = (fg=0, p=0) -> groups 0 (b=0) and 4 (b=1)
            for b in range(B):
                g = b * 4
                for kind, src in ((0, DEST[g][0:1, 0:TSTART]),
                                  (1, EB8[0:1, b, 0, 0:TSTART])):
                    nc.sync.dma_start(
                        bass.AP(tensor=out_head,
                                offset=(kind * B + b) * TSTART,
                                ap=[[TSTART, 1], [1, TSTART]]),
                        src,
                    )
    nc.finalize()
    return nc


def _get_nc():
    if "nc" not in _CACHE:
        _CACHE["nc"] = _build()
    return _CACHE["nc"]


def run_shards(d, x, trace=False, **kw):
    from concourse.bass_utils import run_bass_kernel_spmd

    nc = _get_nc()
    in_maps = []
    for c in range(NC):
        lo = c * CORE_STRIDE
        in_maps.append({
            "x": np.ascontiguousarray(x[lo:lo + SPAN], dtype=np.float32),
            "d": np.ascontiguousarray(d[:, lo:lo + SPAN], dtype=np.float32),
        })
    return run_bass_kernel_spmd(nc, in_maps, core_ids=list(range(NC)),
                                trace=trace, **kw)


def assemble(results):
    mains = np.stack([r["out_main"] for r in results])  # (8, 2, B, 512, 192)
    head = results[0]["out_head"]                       # (2, B, 224)
    outs = []
    for kind in range(2):
        m = mains[:, kind].transpose(1, 0, 2, 3).reshape(B, F, TAIL)
        o = np.zeros((B, OUT_LEN), np.float32)
        o[:, WD:WD + TSTART] = head[kind]
        o[:, WD + TSTART:FRAMELEN - K] = m[:, 0]
        o[:, FRAMELEN - K:] = m[:, 1:].reshape(B, -1)
        outs.append(o)
    return outs[0], outs[1]


def kernel(d, x):
    res = run_shards(d, x)
    return assemble(res.results)
